# revision 4
# baseline (speedup 1.0000x reference)
"""Trainium2 Bass kernel for nn_SGB_A (semi-CRF segmental LM loss).

Strategy: data-parallel over batch B=16 -> 8 cores x 2 sentences.
Per core, on device:
  - encoder LSTM (fwd+bwd stacked, transposed "layout B": gate dims on
    partitions, lanes on free dim), T=128 sequential steps
  - two decoder LSTMs (f/b), 4 steps, 256 rows each, layout B
  - h->vocab projection (V=8000) with fused exp+accumulate logsumexp
  - char/tag logit extraction via gathered-weight row-dots
Host: input embedding prep (index arithmetic + table lookup), final
tiny assembly of P [T,W,B] and the W=3-band semi-CRF scan (~6K flops).

All matmuls bf16 (1 cycle/row on PE); accumulation fp32 in PSUM.
tanh(x) = 2*sigmoid(2x)-1 so one Sigmoid ACT covers all 4 LSTM gates
(g-gate weight rows pre-scaled by 2 on host).
"""

import os
import sys

import numpy as np

for _p in ("/opt/trn_rl_repo",):
    if os.path.isdir(_p) and _p not in sys.path:
        sys.path.insert(0, _p)

import ml_dtypes

BF16 = ml_dtypes.bfloat16
F32 = np.float32

V, D, H, B, T, W = 8000, 256, 256, 16, 128, 3
NCORES, BC = 8, 2
PART = 0
NEG = F32(-1e30)
HK = 2          # 128-partition chunks of H/D
GM = 8          # gate-dim m-chunks (4H/128)
NROW = BC * T   # decoder rows per core (256)
JSZ = 512
NJ = (V + JSZ - 1) // JSZ   # 16 vocab chunks (15x512 + 320)
SEQL = 2 * (T + 1)          # enc h-history cols per k-chunk (258)


def _scale_lstm(Wih, Whh, bih, bhh):
    """Host: fold bih+bhh, pre-scale g-gate rows by 2 (tanh-via-sigmoid)."""
    s = np.ones((4 * H,), F32)
    s[2 * H:3 * H] = 2.0
    WihT = (Wih * s[:, None]).T.astype(F32)   # [D, 4H]
    WhhT = (Whh * s[:, None]).T.astype(F32)   # [H, 4H]
    bias = ((bih + bhh) * s).astype(F32)      # [4H]
    return WihT, WhhT, bias


def _prep_core(c, sentence, emb, enc_p, decf_p, decb_p):
    """Build the per-core input map + host context for core c."""
    sc = sentence[2 * c:2 * c + 2]                      # [2, T]
    fvec, bvec = sc, sc[:, ::-1]
    eWihT, eWhhT, ebias = enc_p

    im = {}
    # --- encoder inputs ------------------------------------------------
    # enc_XT [128, g(2) * k(2) * (T*2)]  col = g*512 + k*256 + t*2 + b
    xt = np.zeros((128, 2 * HK * T * 2), F32)
    for g, vec in enumerate((fvec, bvec)):
        toks = np.concatenate(
            [np.full((2, 1), PART, np.int32), vec[:, :-1]], axis=1)  # [2,T]
        xe = emb[toks]                                   # [2, T, D]
        for k in range(HK):
            blk = xe[:, :, 128 * k:128 * (k + 1)]        # [2, T, 128]
            # col within block = t*2 + b
            xt[:, g * 512 + k * 256: g * 512 + k * 256 + 256] = (
                blk.transpose(2, 1, 0).reshape(128, 256))
    im["enc_XT"] = xt.astype(BF16)
    im["enc_WihT"] = np.concatenate(
        [eWihT[128 * k:128 * (k + 1), :] for k in range(HK)], axis=1
    ).astype(BF16)                                       # [128, 2*1024]
    im["enc_WhhT"] = np.concatenate(
        [eWhhT[128 * k:128 * (k + 1), :] for k in range(HK)], axis=1
    ).astype(BF16)
    im["enc_biasL"] = ebias.reshape(1, 4 * H).astype(BF16)   # [1, 1024]
    im["ident"] = np.eye(128, dtype=F32)

    # --- decoder inputs ------------------------------------------------
    idx = np.clip(np.arange(T)[:, None] + np.arange(W)[None, :], 0, T - 1)
    host = {"wins": [], "bps": []}
    for d, (vec, (WihT, WhhT, bias, Wp, bp)) in enumerate(
            ((fvec, decf_p), (bvec, decb_p))):
        win = vec[:, idx].reshape(BC * T, W)             # rows n=b*T+t
        host["wins"].append(win)
        host["bps"].append(bp)
        toks = np.concatenate(
            [np.full((NROW, 1), PART, np.int32), win], axis=1)  # [256, 4]
        xe = emb[toks]                                   # [256, 4, D]
        # dec_XT [128, k*1024 + s*256 + n]
        xt = np.zeros((128, HK * 4 * 256), F32)
        for k in range(HK):
            blk = xe[:, :, 128 * k:128 * (k + 1)]        # [256,4,128]
            xt[:, k * 1024:(k + 1) * 1024] = (
                blk.transpose(2, 1, 0).reshape(128, 1024))
        im[f"dec_XT_{d}"] = xt.astype(BF16)
        im[f"dec_WihT_{d}"] = np.concatenate(
            [WihT[128 * k:128 * (k + 1), :] for k in range(HK)], axis=1
        ).astype(BF16)
        im[f"dec_WhhT_{d}"] = np.concatenate(
            [WhhT[128 * k:128 * (k + 1), :] for k in range(HK)], axis=1
        ).astype(BF16)
        im[f"dec_biasT_{d}"] = bias.reshape(GM, 128).T.copy().astype(F32)
        WpT = Wp.T.astype(F32)                           # [H, V]
        im[f"WpT_{d}"] = np.concatenate(
            [WpT[128 * k:128 * (k + 1), :] for k in range(HK)], axis=1
        ).astype(BF16)                                   # [128, 2*8000]
        im[f"bpT_{d}"] = bp.reshape(1, V).astype(BF16)
        # gathered Wp rows for char logits: rows (s<3, n) col = s*256+n
        wsel = Wp[win.T.reshape(-1)]                     # [3*256, H] row s*256+n? win.T = [W, N] -> flat s*N+n  ✓
        wselT = wsel.T.astype(F32)                       # [H, 768]
        im[f"WselT_{d}"] = np.concatenate(
            [wselT[128 * k:128 * (k + 1), :] for k in range(HK)], axis=1
        ).astype(BF16)                                   # [128, 2*768]
        im[f"Wp0T_{d}"] = Wp[0].reshape(HK, 128).T.copy().astype(BF16)  # [128,2]
    return im, host


def build_nc():
    import concourse.bacc as bacc
    import concourse.mybir as mybir
    import concourse.tile as tile

    dt = mybir.dt
    AF = mybir.ActivationFunctionType
    OP = mybir.AluOpType

    nc = bacc.Bacc(None, target_bir_lowering=False)

    # ---- dram I/O -----------------------------------------------------
    di = {}
    def inp(name, shape, dtype=dt.bfloat16):
        di[name] = nc.dram_tensor(name, shape, dtype, kind="ExternalInput")
    inp("enc_XT", [128, 1024])
    inp("enc_WihT", [128, 2048])
    inp("enc_WhhT", [128, 2048])
    inp("enc_biasL", [1, 1024])
    inp("ident", [128, 128], dt.float32)
    for d in range(2):
        inp(f"dec_XT_{d}", [128, 2048])
        inp(f"dec_WihT_{d}", [128, 2048])
        inp(f"dec_WhhT_{d}", [128, 2048])
        inp(f"dec_biasT_{d}", [128, GM], dt.float32)
        inp(f"WpT_{d}", [128, 2 * V])
        inp(f"bpT_{d}", [1, V])
        inp(f"WselT_{d}", [128, 2 * 768])
        inp(f"Wp0T_{d}", [128, 2])
    do = {}
    for d in range(2):
        do[f"logZ_{d}"] = nc.dram_tensor(
            f"logZ_{d}", [128, GM], dt.float32, kind="ExternalOutput")
        do[f"char_{d}"] = nc.dram_tensor(
            f"char_{d}", [1, 768], dt.float32, kind="ExternalOutput")
        do[f"tag_{d}"] = nc.dram_tensor(
            f"tag_{d}", [1, 768], dt.float32, kind="ExternalOutput")

    with tile.TileContext(nc) as tc:
        with (
            tc.tile_pool(name="wts", bufs=1) as wts,
            tc.tile_pool(name="state", bufs=1) as state,
        ):
            # ---- load everything -------------------------------------
            sb = {}
            for name, dr in di.items():
                t = wts.tile(list(dr.shape), dr.dtype, tag=name)
                nc.sync.dma_start(t[:], dr[:])
                sb[name] = t

            ones_r = wts.tile([1, 256], dt.bfloat16, tag="ones_r")
            nc.vector.memset(ones_r[:], 1.0)
            ones_col = wts.tile([128, 1], dt.bfloat16, tag="ones_col")
            nc.vector.memset(ones_col[:], 1.0)

            def blk(t, k, m, width=1024):
                return t[:, k * width + 128 * m: k * width + 128 * (m + 1)]

            # ---- encoder xWb precompute ------------------------------
            # xWbT [128, g*2048 + m*256 + t*2 + b] fp32
            enc_scope = tc.tile_pool(name="encp", bufs=1)
            encp = enc_scope.__enter__()
            escr_scope = tc.tile_pool(name="escr", bufs=3)
            escr = escr_scope.__enter__()
            xwb = encp.tile([128, 2 * GM * 256], dt.float32, tag="xwb")
            with tc.tile_pool(name="ps_pre", bufs=4, space="PSUM") as ps_pre:
                for g in range(2):
                    for m in range(GM):
                        px = ps_pre.tile([128, 256], dt.float32, tag="px")
                        for k in range(HK):
                            nc.tensor.matmul(
                                px[:], blk(sb["enc_WihT"], k, m),
                                sb["enc_XT"][:, g * 512 + k * 256:
                                             g * 512 + (k + 1) * 256],
                                start=(k == 0), stop=False,
                                skip_group_check=True)
                        nc.tensor.matmul(
                            px[:], sb["enc_biasL"][:, 128 * m:128 * (m + 1)],
                            ones_r[:], start=False, stop=True,
                            skip_group_check=True)
                        nc.vector.tensor_copy(
                            xwb[:, g * 2048 + m * 256:g * 2048 + (m + 1) * 256],
                            px[:])

            # ---- encoder recurrence ----------------------------------
            # hsT per g: [128, k*SEQL + t*2 + b] bf16 (t=0 is h0=0)
            hs_enc = []
            for g in range(2):
                hst = state.tile([128, HK * SEQL], dt.bfloat16, tag=f"hst{g}")
                nc.vector.memset(hst[:, 0:2], 0.0)
                nc.vector.memset(hst[:, SEQL:SEQL + 2], 0.0)
                hs_enc.append(hst)
            ct_enc = []
            for g in range(2):
                ct = state.tile([128, HK * 2], dt.float32, tag=f"ct{g}")
                nc.vector.memset(ct[:], 0.0)
                ct_enc.append(ct)

            xwb_r = xwb[:].rearrange("p (g m c) -> p g m c", g=2, m=GM)

            with tc.tile_pool(name="ps_enc", bufs=4, space="PSUM") as ps_enc:
                for t in range(T):
                    for g in range(2):
                        hst, ct = hs_enc[g], ct_enc[g]
                        pe = ps_enc.tile([128, 16], dt.float32, tag="pe")
                        # inject xWb[t] via identity matmul (start=True)
                        nc.tensor.matmul(
                            pe[:], sb["ident"][:],
                            xwb_r[:, g, :, 2 * t:2 * t + 2],
                            start=True, stop=False, skip_group_check=True)
                        for m in range(GM):
                            for k in range(HK):
                                nc.tensor.matmul(
                                    pe[:, 2 * m:2 * m + 2],
                                    blk(sb["enc_WhhT"], k, m),
                                    hst[:, k * SEQL + 2 * t:
                                        k * SEQL + 2 * t + 2],
                                    start=False,
                                    stop=(m == GM - 1 and k == HK - 1),
                                    skip_group_check=True)
                        sg = escr.tile([128, 16], dt.float32, tag="sg")
                        nc.scalar.activation(sg[:], pe[:], AF.Sigmoid)
                        tg = escr.tile([128, 4], dt.float32, tag="tg")
                        nc.vector.tensor_scalar(
                            tg[:], sg[:, 8:12], 2.0, -1.0,
                            OP.mult, OP.add)
                        t1 = escr.tile([128, 4], dt.float32, tag="t1")
                        nc.vector.tensor_tensor(
                            t1[:], sg[:, 0:4], tg[:], OP.mult)
                        t2 = escr.tile([128, 4], dt.float32, tag="t2")
                        nc.vector.tensor_tensor(
                            t2[:], sg[:, 4:8], ct[:], OP.mult)
                        nc.vector.tensor_tensor(
                            ct[:], t1[:], t2[:], OP.add)
                        th = escr.tile([128, 4], dt.float32, tag="th")
                        nc.scalar.activation(th[:], ct[:], AF.Tanh)
                        hout = hst[:].rearrange(
                            "p (k c) -> p k c", k=HK)[
                            :, :, 2 * (t + 1):2 * (t + 1) + 2]
                        nc.vector.tensor_tensor(
                            hout, sg[:, 12:16], th[:], OP.mult)

            # ---- decoders --------------------------------------------
            # hsdT per d: [128, k*1280 + s*256 + n] bf16, slot 0 = h0
            escr_scope.__exit__(None, None, None)
            enc_scope.__exit__(None, None, None)
            outs = {}
            with (
                tc.tile_pool(name="dscr", bufs=2) as dscr,
                tc.tile_pool(name="sgp", bufs=1) as sgp,
                tc.tile_pool(name="ps_dec", bufs=1, space="PSUM") as ps_dec,
                tc.tile_pool(name="ps_proj", bufs=2, space="PSUM") as ps_proj,
                tc.tile_pool(name="ps_ct", bufs=1, space="PSUM") as ps_ct,
            ):
                for d in range(2):
                    hst = hs_enc[d]
                    hsd = state.tile([128, HK * 5 * 256], dt.bfloat16,
                                     tag=f"hsd{d}")
                    cd = state.tile([128, HK * 256], dt.float32, tag=f"cd{d}")
                    hsr = hst[:].rearrange("p (k s) -> p k s", k=HK)
                    for k in range(HK):
                        # src col = 2(t+1)+b = 2t + 2 + b ; dst col = b*128+t
                        src = hsr[:, k, 2:2 + 256].rearrange(
                            "p (t b) -> p b t", b=2)
                        nc.vector.tensor_copy(
                            hsd[:, k * 1280:k * 1280 + 256], src)
                        nc.vector.tensor_copy(
                            cd[:, k * 256:(k + 1) * 256], src)

                    WhhT, WihT = sb[f"dec_WhhT_{d}"], sb[f"dec_WihT_{d}"]
                    for s in range(4):
                        pd = ps_dec.tile([128, GM * 256], dt.float32,
                                         tag="pd")
                        for m in range(GM):
                            for k in range(HK):
                                nc.tensor.matmul(
                                    pd[:, m * 256:(m + 1) * 256],
                                    blk(WhhT, k, m),
                                    hsd[:, k * 1280 + s * 256:
                                        k * 1280 + (s + 1) * 256],
                                    start=(k == 0), stop=False,
                                    skip_group_check=True)
                            for k in range(HK):
                                nc.tensor.matmul(
                                    pd[:, m * 256:(m + 1) * 256],
                                    blk(WihT, k, m),
                                    sb[f"dec_XT_{d}"][
                                        :, k * 1024 + s * 256:
                                        k * 1024 + (s + 1) * 256],
                                    start=False, stop=(k == HK - 1),
                                    skip_group_check=True)
                        sgd = sgp.tile([128, GM * 256], dt.float32,
                                        tag="sgd")
                        for m in range(GM):
                            nc.scalar.activation(
                                sgd[:, m * 256:(m + 1) * 256],
                                pd[:, m * 256:(m + 1) * 256],
                                AF.Sigmoid,
                                bias=sb[f"dec_biasT_{d}"][:, m:m + 1])
                        tgd = dscr.tile([128, 512], dt.float32, tag="tgd")
                        nc.vector.tensor_scalar(
                            tgd[:], sgd[:, 1024:1536], 2.0, -1.0,
                            OP.mult, OP.add)
                        t1d = dscr.tile([128, 512], dt.float32, tag="t1d")
                        nc.vector.tensor_tensor(
                            t1d[:], sgd[:, 0:512], tgd[:], OP.mult)
                        t2d = dscr.tile([128, 512], dt.float32, tag="t2d")
                        nc.vector.tensor_tensor(
                            t2d[:], sgd[:, 512:1024], cd[:], OP.mult)
                        nc.vector.tensor_tensor(
                            cd[:], t1d[:], t2d[:], OP.add)
                        thd = dscr.tile([128, 512], dt.float32, tag="thd")
                        nc.scalar.activation(thd[:], cd[:], AF.Tanh)
                        hdo = hsd[:].rearrange(
                            "p (k s) -> p k s", k=HK)[
                            :, :, (s + 1) * 256:(s + 2) * 256]
                        nc.vector.tensor_tensor(
                            hdo, sgd[:, 1536:2048], thd[:], OP.mult)

                    # ---- projection + logsumexp ----------------------
                    sums = state.tile([128, GM * NJ], dt.float32,
                                      tag=f"sums{d}")
                    WpT, bpT = sb[f"WpT_{d}"], sb[f"bpT_{d}"]
                    for r in range(GM):
                        s, half = 1 + r // 2, r % 2
                        for j in range(NJ):
                            nj = min(JSZ, V - j * JSZ)
                            pp = ps_proj.tile([128, JSZ], dt.float32,
                                              tag="pp")
                            for k in range(HK):
                                nc.tensor.matmul(
                                    pp[:, 0:nj],
                                    hsd[:, k * 1280 + s * 256 + half * 128:
                                        k * 1280 + s * 256 + half * 128 + 128],
                                    WpT[:, k * V + j * JSZ:
                                        k * V + j * JSZ + nj],
                                    start=(k == 0), stop=False,
                                    skip_group_check=True)
                            nc.tensor.matmul(
                                pp[:, 0:nj], ones_r[:, 0:128],
                                bpT[:, j * JSZ:j * JSZ + nj],
                                start=False, stop=True,
                                skip_group_check=True)
                            ex = dscr.tile([128, JSZ], dt.bfloat16,
                                           tag="ex")
                            nc.scalar.activation(
                                ex[:, 0:nj], pp[:, 0:nj], AF.Exp,
                                accum_out=sums[:, r * NJ + j:r * NJ + j + 1])
                    # logZ
                    lz = state.tile([128, GM], dt.float32, tag=f"lz{d}")
                    junk = dscr.tile([128, NJ], dt.float32, tag="junk")
                    zs = dscr.tile([128, 1], dt.float32, tag="zs")
                    for r in range(GM):
                        nc.vector.tensor_scalar(
                            junk[:], sums[:, r * NJ:(r + 1) * NJ],
                            1.0, 0.0, OP.mult, OP.add,
                            accum_out=zs[:])
                        nc.scalar.activation(
                            lz[:, r:r + 1], zs[:], AF.Ln)
                    nc.sync.dma_start(do[f"logZ_{d}"][:], lz[:])

                    # ---- char / tag dots -----------------------------
                    muls = sgp.tile([128, HK * 768], dt.bfloat16,
                                     tag="muls")
                    for k in range(HK):
                        nc.vector.tensor_tensor(
                            muls[:, k * 768:(k + 1) * 768],
                            hsd[:, k * 1280 + 256:k * 1280 + 1024],
                            sb[f"WselT_{d}"][:, k * 768:(k + 1) * 768],
                            OP.mult)
                    pc1 = ps_ct.tile([1, 512], dt.float32, tag="pc1")
                    pc2 = ps_ct.tile([1, 256], dt.float32, tag="pc2")
                    for k in range(HK):
                        nc.tensor.matmul(
                            pc1[:], ones_col[:],
                            muls[:, k * 768:k * 768 + 512],
                            start=(k == 0), stop=(k == HK - 1),
                            skip_group_check=True)
                        nc.tensor.matmul(
                            pc2[:], ones_col[:],
                            muls[:, k * 768 + 512:(k + 1) * 768],
                            start=(k == 0), stop=(k == HK - 1),
                            skip_group_check=True)
                    csb = state.tile([1, 768], dt.float32, tag=f"csb{d}")
                    nc.vector.tensor_copy(csb[:, 0:512], pc1[:])
                    nc.vector.tensor_copy(csb[:, 512:768], pc2[:])
                    nc.sync.dma_start(do[f"char_{d}"][:], csb[:])

                    pt1 = ps_ct.tile([1, 512], dt.float32, tag="pc1")
                    pt2 = ps_ct.tile([1, 256], dt.float32, tag="pc2")
                    for k in range(HK):
                        nc.tensor.matmul(
                            pt1[:], sb[f"Wp0T_{d}"][:, k:k + 1],
                            hsd[:, k * 1280 + 512:k * 1280 + 1024],
                            start=(k == 0), stop=(k == HK - 1),
                            skip_group_check=True)
                        nc.tensor.matmul(
                            pt2[:], sb[f"Wp0T_{d}"][:, k:k + 1],
                            hsd[:, k * 1280 + 1024:k * 1280 + 1280],
                            start=(k == 0), stop=(k == HK - 1),
                            skip_group_check=True)
                    tsb = state.tile([1, 768], dt.float32, tag=f"tsb{d}")
                    nc.vector.tensor_copy(tsb[:, 0:512], pt1[:])
                    nc.vector.tensor_copy(tsb[:, 512:768], pt2[:])
                    nc.sync.dma_start(do[f"tag_{d}"][:], tsb[:])
                    outs[d] = True

    nc.compile()
    return nc


def _postprocess(core_outs, hosts):
    """core_outs: list of 8 dicts with logZ_d [128,8], char_d/tag_d [1,768].
    Returns scalar loss (f32)."""
    Ps = []
    for c in range(NCORES):
        o, host = core_outs[c], hosts[c]
        decP = []
        for d in range(2):
            logZ = np.asarray(o[f"logZ_{d}"], F32).T.reshape(4 * 256)
            logZ = logZ.reshape(4, 256)
            win = hosts[c]["wins"][d]              # [256, W]
            bp = hosts[c]["bps"][d]
            cdot = np.asarray(o[f"char_{d}"], F32).reshape(3, 256)
            tdot = np.asarray(o[f"tag_{d}"], F32).reshape(3, 256)
            char = cdot + bp[win.T] - logZ[:3]
            tag = tdot + bp[0] - logZ[1:]
            p = np.cumsum(char, axis=0) + tag       # [W, 256]
            decP.append(p.reshape(W, BC, T))
        fwdP = decP[0].transpose(2, 0, 1)          # [T, W, BC]
        bwdP = decP[1].transpose(2, 0, 1)
        xs = np.arange(T)[:, None]
        ys = np.arange(W)[None, :]
        Pm = 0.5 * (fwdP[np.clip(xs - ys, 0, T - 1), ys]
                    + bwdP[T - 1 - xs, ys])         # [T, W, BC]
        Ps.append(Pm.astype(F32))
    P = np.concatenate(Ps, axis=2)                 # [T, W, B]

    buf = np.zeros((W, B), F32)
    for j in range(1, T + 1):
        p_j = P[j - 1]
        cand = np.where((np.arange(W) < j)[:, None], buf + p_j, NEG)
        mx = cand.max(axis=0)
        tot = (np.log(np.exp(cand - mx).sum(axis=0)) + mx).astype(F32)
        buf = np.concatenate([tot[None], buf[:-1]], axis=0)
    return F32(-np.mean(buf[0]))


def _prep_all(sentence, emb, enc_p, decf_p, decb_p):
    in_maps, hosts = [], []
    for c in range(NCORES):
        im, host = _prep_core(c, sentence, emb, enc_p, decf_p, decb_p)
        in_maps.append(im)
        hosts.append(host)
    return in_maps, hosts


_NC_CACHE = []


def kernel(sentence, emb, enc_Wih, enc_Whh, enc_bih, enc_bhh,
           decf_Wih, decf_Whh, decf_bih, decf_bhh, decf_Wp, decf_bp,
           decb_Wih, decb_Whh, decb_bih, decb_bhh, decb_Wp, decb_bp):
    sentence = np.asarray(sentence, np.int32)
    emb = np.asarray(emb, F32)
    enc_p = _scale_lstm(enc_Wih, enc_Whh, enc_bih, enc_bhh)
    decf_p = _scale_lstm(decf_Wih, decf_Whh, decf_bih, decf_bhh) + (
        np.asarray(decf_Wp, F32), np.asarray(decf_bp, F32))
    decb_p = _scale_lstm(decb_Wih, decb_Whh, decb_bih, decb_bhh) + (
        np.asarray(decb_Wp, F32), np.asarray(decb_bp, F32))
    decf_p = (decf_p[0], decf_p[1], decf_p[2], decf_p[3], decf_p[4])
    decb_p = (decb_p[0], decb_p[1], decb_p[2], decb_p[3], decb_p[4])
    in_maps, hosts = _prep_all(sentence, emb, enc_p, decf_p, decb_p)

    from concourse.bass_utils import run_bass_kernel_spmd
    if not _NC_CACHE:
        _NC_CACHE.append(build_nc())
    nc = _NC_CACHE[0]
    res = run_bass_kernel_spmd(nc, in_maps, list(range(NCORES)))
    return np.asarray(_postprocess(res.results, hosts), F32)


# ---------------------------------------------------------------------
# numpy emulation of the device program (for math validation, no HW)
# ---------------------------------------------------------------------
def _emulate_core(im, host):
    def f(x):
        return np.asarray(x, F32)

    out = {}
    # encoder
    WihT = f(im["enc_WihT"])
    WhhT = f(im["enc_WhhT"])
    biasL = f(im["enc_biasL"])[0]
    xT = f(im["enc_XT"])
    hs_enc = []
    for g in range(2):
        xwb = np.zeros((128, GM * 256), F32)
        for m in range(GM):
            acc = np.zeros((128, 256), F32)
            for k in range(HK):
                lhsT = WhhT[:, 0:0]  # placeholder
                lhsT = WihT[:, k * 1024 + 128 * m:k * 1024 + 128 * (m + 1)]
                rhs = xT[:, g * 512 + k * 256:g * 512 + (k + 1) * 256]
                acc += lhsT.T @ rhs
            acc += biasL[128 * m:128 * (m + 1)][:, None]
            xwb[:, m * 256:(m + 1) * 256] = acc
        hst = np.zeros((128, HK * SEQL), F32)
        ct = np.zeros((128, HK * 2), F32)
        for t in range(T):
            pe = xwb.reshape(128, GM, 256)[
                :, :, 2 * t:2 * t + 2].reshape(128, 16).copy()
            for m in range(GM):
                for k in range(HK):
                    lhsT = WhhT[:, k * 1024 + 128 * m:k * 1024 + 128 * (m + 1)]
                    rhs = hst[:, k * SEQL + 2 * t:k * SEQL + 2 * t + 2]
                    pe[:, 2 * m:2 * m + 2] += (
                        lhsT.astype(BF16).astype(F32).T
                        @ rhs.astype(BF16).astype(F32))
            sg = 1.0 / (1.0 + np.exp(-pe))
            tg = sg[:, 8:12] * 2.0 - 1.0
            ct = sg[:, 0:4] * tg + sg[:, 4:8] * ct
            th = np.tanh(ct)
            h = sg[:, 12:16] * th
            for k in range(HK):
                hst[:, k * SEQL + 2 * (t + 1):k * SEQL + 2 * (t + 1) + 2] = (
                    h[:, 2 * k:2 * k + 2])
        hs_enc.append(hst)

    for d in range(2):
        WihTd = f(im[f"dec_WihT_{d}"])
        WhhTd = f(im[f"dec_WhhT_{d}"])
        biasT = f(im[f"dec_biasT_{d}"])
        xTd = f(im[f"dec_XT_{d}"])
        hst = hs_enc[d]
        hsd = np.zeros((128, HK * 5 * 256), F32)
        cd = np.zeros((128, HK * 256), F32)
        for k in range(HK):
            src = hst[:, k * SEQL + 2:k * SEQL + 2 + 256].reshape(
                128, 128, 2).transpose(0, 2, 1).reshape(128, 256)
            hsd[:, k * 1280:k * 1280 + 256] = src
            cd[:, k * 256:(k + 1) * 256] = src
        for s in range(4):
            pd = np.zeros((128, GM * 256), F32)
            for m in range(GM):
                for k in range(HK):
                    lhsT = WhhTd[:, k * 1024 + 128 * m:
                                 k * 1024 + 128 * (m + 1)]
                    rhs = hsd[:, k * 1280 + s * 256:k * 1280 + (s + 1) * 256]
                    pd[:, m * 256:(m + 1) * 256] += (
                        lhsT.astype(BF16).astype(F32).T
                        @ rhs.astype(BF16).astype(F32))
                    lhsT = WihTd[:, k * 1024 + 128 * m:
                                k * 1024 + 128 * (m + 1)]
                    rhs = xTd[:, k * 1024 + s * 256:k * 1024 + (s + 1) * 256]
                    pd[:, m * 256:(m + 1) * 256] += lhsT.T @ rhs
                pd[:, m * 256:(m + 1) * 256] += biasT[:, m:m + 1]
            sgd = 1.0 / (1.0 + np.exp(-pd))
            tgd = sgd[:, 1024:1536] * 2.0 - 1.0
            cd = sgd[:, 0:512] * tgd + sgd[:, 512:1024] * cd
            thd = np.tanh(cd)
            h = sgd[:, 1536:2048] * thd
            for k in range(HK):
                hsd[:, k * 1280 + (s + 1) * 256:k * 1280 + (s + 2) * 256] = (
                    h[:, k * 256:(k + 1) * 256])
        # projection
        WpT = f(im[f"WpT_{d}"])
        bpT = f(im[f"bpT_{d}"])[0]
        sums = np.zeros((128, GM, NJ), F32)
        for r in range(GM):
            s, half = 1 + r // 2, r % 2
            lhs = np.concatenate(
                [hsd[:, k * 1280 + s * 256 + half * 128:
                     k * 1280 + s * 256 + half * 128 + 128]
                 for k in range(HK)], axis=0)          # [256, 128]
            for j in range(NJ):
                nj = min(JSZ, V - j * JSZ)
                rhs = np.concatenate(
                    [WpT[:, k * V + j * JSZ:k * V + j * JSZ + nj]
                     for k in range(HK)], axis=0)
                logits = (lhs.astype(BF16).astype(F32).T
                          @ rhs.astype(BF16).astype(F32)
                          + bpT[j * JSZ:j * JSZ + nj][None, :])
                sums[:, r, j] = np.exp(logits).sum(axis=1)
        lz = np.log(sums.sum(axis=2))                  # [128, GM]
        out[f"logZ_{d}"] = lz
        # char/tag
        WselT = f(im[f"WselT_{d}"])
        Wp0T = f(im[f"Wp0T_{d}"])
        cdot = np.zeros((768,), F32)
        tdot = np.zeros((768,), F32)
        for k in range(HK):
            hslice = hsd[:, k * 1280 + 256:k * 1280 + 1024]
            cdot += (hslice.astype(BF16).astype(F32)
                     * WselT[:, k * 768:(k + 1) * 768]).sum(axis=0)
            tslice = hsd[:, k * 1280 + 512:k * 1280 + 1280]
            tdot += (Wp0T[:, k:k + 1]
                     * tslice.astype(BF16).astype(F32)).sum(axis=0)
        out[f"char_{d}"] = cdot.reshape(1, 768)
        out[f"tag_{d}"] = tdot.reshape(1, 768)
    return out


def kernel_emulated(**inputs):
    sentence = np.asarray(inputs["sentence"], np.int32)
    emb = np.asarray(inputs["emb"], F32)
    enc_p = _scale_lstm(inputs["enc_Wih"], inputs["enc_Whh"],
                        inputs["enc_bih"], inputs["enc_bhh"])
    decf_p = _scale_lstm(inputs["decf_Wih"], inputs["decf_Whh"],
                         inputs["decf_bih"], inputs["decf_bhh"]) + (
        np.asarray(inputs["decf_Wp"], F32), np.asarray(inputs["decf_bp"], F32))
    decb_p = _scale_lstm(inputs["decb_Wih"], inputs["decb_Whh"],
                         inputs["decb_bih"], inputs["decb_bhh"]) + (
        np.asarray(inputs["decb_Wp"], F32), np.asarray(inputs["decb_bp"], F32))
    in_maps, hosts = _prep_all(sentence, emb, enc_p, decf_p, decb_p)
    core_outs = [_emulate_core(im, host) for im, host in
                 zip(in_maps, hosts)]
    return _postprocess(core_outs, hosts)
